# revision 16
# baseline (speedup 1.0000x reference)
"""Grouped-expert SwiGLU FFN (MoE) kernel for 8 Trainium2 NeuronCores.

Problem: 8 experts, tokens pre-sorted into contiguous equal segments.
  sorted_x: (8192, 512) f32, w12: (8, 2048, 512) f32, w3: (8, 512, 1024) f32
  out[t] = SwiGLU(x[t] @ w12[e].T) @ w3[e].T   for t in expert e's segment

Sharding: expert parallelism. Core e gets its 1024-token segment plus
w12[e]/w3[e]; no collectives. Host pre-transposes operands so the
contraction dim lands on SBUF partitions (no on-chip transposes):

  GEMM1 (contract d_model=512, 4 chunks of 128):
    lhsT = w12[e].T chunk (i=128, j=128)  [stationary]
    rhs  = x_seg.T  chunk (i=128, t=512)  [moving]
    psum (j=128, t=512) accumulated over 4 i-chunks  -> h12^T orientation
  SwiGLU: silu(h1^T) * h2^T elementwise in (j, t) layout (ACT + DVE)
  GEMM2 (contract hidden=1024, 8 chunks of 128):
    lhsT = h^T chunk   (j=128, t=128)  [stationary]
    rhs  = w3[e].T chunk (j=128, o=512) [moving]
    psum (t=128, o=512) accumulated over 8 j-chunks -> natural (t, o) output

All operands are cast to fp16 on the host: halves HBM traffic vs f32 and
enables Fast Weight Load (disabled for fp32 dtypes), so LDWEIGHTS hides
under the 512-col matmul streams. PSUM accumulation stays fp32. Inputs
arrive in 6 host-packed partition-major DMAs ordered by consumption so
the first matmul is gated by only ~768 KB (w12 pair 0 + x token-block 0)
instead of the whole 4 MB. GEMM2 token-chunks are interleaved between
the two GEMM1 token-block passes so the SwiGLU ACT/DVE latency of one
block hides under the other block's matmuls. The output is stored fp16
(host upcasts) to shorten the store tail.
"""

import numpy as np

N_EXPERTS = 8
D_MODEL = 512
HIDDEN = 1024
TOKENS_PER_EXPERT = 1024
N_CORES = 8

_CACHE = {}


def _build_program(warm_mms=18):
    import concourse.mybir as mybir
    import concourse.tile as tile
    from concourse import bacc

    f32 = mybir.dt.float32
    f16 = mybir.dt.float16
    P = 128
    IC = D_MODEL // P            # 4 contraction chunks of d_model
    JC = HIDDEN // P             # 8 pair-chunks of 2*hidden
    TB = 512                     # token block (moving free dim / psum bank)
    NT = TOKENS_PER_EXPERT // TB  # 2 token blocks
    TM = TOKENS_PER_EXPERT // P  # 8 output token chunks

    nc = bacc.Bacc(None, target_bir_lowering=False)

    # host-packed partition-major layouts (see _pack_inputs)
    xt_d = nc.dram_tensor("xt", [NT, P, IC, TB], f16, kind="ExternalInput")
    w12a_d = nc.dram_tensor("w12a", [P, 2, IC * P], f16, kind="ExternalInput")
    w12b_d = nc.dram_tensor("w12b", [P, JC - 1, 2, IC * P], f16, kind="ExternalInput")
    w3_d = nc.dram_tensor("w3g", [P, JC, D_MODEL], f16, kind="ExternalInput")
    out_d = nc.dram_tensor("out", [TOKENS_PER_EXPERT, D_MODEL], f16,
                           kind="ExternalOutput")

    with tile.TileContext(nc) as tc:
        with (
            tc.tile_pool(name="persist", bufs=1) as persist,
            tc.tile_pool(name="work", bufs=3) as work,
            tc.tile_pool(name="ps1", bufs=2, space="PSUM") as ps1,
            tc.tile_pool(name="ps2", bufs=2, space="PSUM") as ps2,
        ):
            # ---- PE warm-up: zero matmuls during the initial DMA window
            # release the HAM clock gate before real work arrives.
            # memset on GpSimd (free right after the preamble) so the
            # dummy matmuls start ~6.3us, not after DVE's late start. ----
            if warm_mms:
                wz = persist.tile([P, 256], f16, tag="warmz", name="warmz")
                nc.gpsimd.memset(wz[:], 0.0)
                wps = ps2.tile([P, 256], f32, tag="warmps", bufs=1, name="warmps")
                for i in range(warm_mms):
                    nc.tensor.matmul(wps[:], wz[:, 0:P], wz[:], start=True,
                                     stop=True)

            # ---- loads, in consumption order. The SP HWDGE ring's
            # per-queue FIFO keeps later loads from stealing bandwidth
            # ahead of their turn; the small gating w12 pair-0 load rides
            # the ACT ring in parallel. ----
            w12a = persist.tile([P, 2, IC * P], f16, tag="w12a", name="w12a")
            nc.scalar.dma_start(w12a[:], w12a_d[:])
            xt = [persist.tile([P, IC, TB], f16, tag=f"xt{tb}", name=f"xt{tb}") for tb in range(NT)]
            nc.sync.dma_start(xt[0][:, 0:2, :], xt_d[0, :, 0:2])
            nc.sync.dma_start(xt[0][:, 2:4, :], xt_d[0, :, 2:4])
            w12b = persist.tile([P, JC - 1, 2, IC * P], f16, tag="w12b", name="w12b")
            nc.sync.dma_start(w12b[:, 0], w12b_d[:, 0])
            nc.sync.dma_start(w12b[:, 1], w12b_d[:, 1])
            nc.sync.dma_start(w12b[:, 2:4], w12b_d[:, 2:4])
            nc.sync.dma_start(w12b[:, 4:6], w12b_d[:, 4:6])
            nc.sync.dma_start(w12b[:, 6], w12b_d[:, 6])
            nc.sync.dma_start(xt[1][:], xt_d[1])
            w3 = persist.tile([P, JC, D_MODEL], f16, tag="w3", name="w3")
            nc.sync.dma_start(w3[:], w3_d[:])

            def w12sel(p, h, ic):
                sl = slice(ic * P, (ic + 1) * P)
                if p == 0:
                    return w12a[:, h, sl]
                return w12b[:, p - 1, h, sl]

            h = [persist.tile([P, TOKENS_PER_EXPERT], f16, tag=f"h{p}", name=f"h{p}")
                 for p in range(JC)]

            def gemm1(tb):
                for p in range(JC):
                    ps_a = ps1.tile([P, TB], f32, tag="ps_a", name=f"a{p}_{tb}")
                    ps_b = ps1.tile([P, TB], f32, tag="ps_b", name=f"b{p}_{tb}")
                    for ic in range(IC):
                        nc.tensor.matmul(ps_a[:], w12sel(p, 0, ic),
                                         xt[tb][:, ic, :],
                                         start=(ic == 0), stop=(ic == IC - 1))
                    for ic in range(IC):
                        nc.tensor.matmul(ps_b[:], w12sel(p, 1, ic),
                                         xt[tb][:, ic, :],
                                         start=(ic == 0), stop=(ic == IC - 1))
                    s = work.tile([P, TB], f32, tag="silu", name=f"s{p}_{tb}")
                    nc.scalar.activation(s[:], ps_a[:],
                                         mybir.ActivationFunctionType.Silu)
                    nc.vector.tensor_tensor(h[p][:, tb * TB:(tb + 1) * TB],
                                            s[:], ps_b[:], mybir.AluOpType.mult)

            def gemm2(tm, split=1):
                # split > 1 divides the psum group into column halves so
                # the final chunk's cast+store chain starts as soon as its
                # half of the matmuls lands, and the stores fan out over
                # both HWDGE rings — shortest exposed tail after the last
                # matmul.
                w = D_MODEL // split
                o = work.tile([P, D_MODEL], f16, tag="o", name=f"oo{tm}")
                for sp in range(split):
                    sl = slice(sp * w, (sp + 1) * w)
                    ps_o = ps2.tile([P, w], f32, tag="ps_o", name=f"o{tm}_{sp}")
                    for jc in range(JC):
                        nc.tensor.matmul(ps_o[:], h[jc][:, tm * P:(tm + 1) * P],
                                         w3[:, jc, sl],
                                         start=(jc == 0), stop=(jc == JC - 1))
                    nc.vector.tensor_copy(o[:, sl], ps_o[:])
                    eng = nc.scalar if sp % 2 else nc.sync
                    eng.dma_start(out_d[tm * P:(tm + 1) * P, sl], o[:, sl])

            gemm1(0)
            gemm1(1)
            for tm in range(TM):
                gemm2(tm, split=2 if tm == TM - 1 else 1)

    nc.compile()
    return nc


def _pack_inputs(sorted_x, w12, w3, starts, per):
    """Host-side shard + transpose + fp16 packing for each core."""
    in_maps = []
    for e in range(N_EXPERTS):
        xs = sorted_x[starts[e]:starts[e] + per]                 # (1024, 512)
        # xt[tb, p, ic, t] = xs[tb*512 + t, ic*128 + p]
        xt = np.ascontiguousarray(
            xs.reshape(2, 512, 4, 128).transpose(0, 3, 2, 1)
        ).astype(np.float16)
        # w12c[q][k][ic*128+j] = w12[e][q*128+j, ic*128+k]
        w12c = w12[e].reshape(16, 128, 4, 128).transpose(0, 3, 2, 1).reshape(
            16, 128, 512)
        # pair-major: pair p = (chunk p [h1], chunk p+8 [h2])
        w12p = np.stack([w12c[:8], w12c[8:]], axis=1)            # (8, 2, 128, 512)
        w12a = np.ascontiguousarray(
            w12p[0].transpose(1, 0, 2)).astype(np.float16)       # (128, 2, 512)
        w12b = np.ascontiguousarray(
            w12p[1:].transpose(2, 0, 1, 3)).astype(np.float16)   # (128, 7, 2, 512)
        # w3g[p][jc][o] = w3[e][o, jc*128+p]
        w3g = np.ascontiguousarray(
            w3[e].reshape(512, 8, 128).transpose(2, 1, 0)
        ).astype(np.float16)                                     # (128, 8, 512)
        in_maps.append({"xt": xt, "w12a": w12a, "w12b": w12b, "w3g": w3g})
    return in_maps


def _reference_numpy(sorted_x, w12, w3, expert_starts, expert_ends):
    """Exact fallback for non-canonical segment layouts."""
    x = sorted_x.astype(np.float32)
    T = x.shape[0]
    out = np.zeros((T, w3.shape[1]), dtype=np.float32)
    tok = np.arange(T)
    for e in range(w12.shape[0]):
        m = (tok >= expert_starts[e]) & (tok < expert_ends[e])
        if not m.any():
            continue
        h12 = x[m] @ w12[e].T
        h1, h2 = h12[:, :HIDDEN], h12[:, HIDDEN:]
        hact = (h1 / (1.0 + np.exp(-h1))) * h2
        out[m] += hact @ w3[e].T
    return out


def kernel(sorted_x, w12, w3, expert_starts, expert_ends):
    sorted_x = np.asarray(sorted_x)
    w12 = np.asarray(w12)
    w3 = np.asarray(w3)
    starts = np.asarray(expert_starts).astype(np.int64)
    ends = np.asarray(expert_ends).astype(np.int64)

    T = sorted_x.shape[0]
    E = w12.shape[0]
    per = T // E
    canonical = (
        E == N_EXPERTS
        and T == N_EXPERTS * TOKENS_PER_EXPERT
        and sorted_x.shape[1] == D_MODEL
        and w12.shape[1:] == (2 * HIDDEN, D_MODEL)
        and w3.shape[1:] == (D_MODEL, HIDDEN)
        and bool(np.all(starts == np.arange(E, dtype=np.int64) * per))
        and bool(np.all(ends == starts + per))
    )
    if not canonical:
        return _reference_numpy(sorted_x, w12, w3, starts, ends)

    from concourse.bass_utils import run_bass_kernel_spmd

    if "nc" not in _CACHE:
        _CACHE["nc"] = _build_program()
    nc = _CACHE["nc"]

    in_maps = _pack_inputs(sorted_x, w12, w3, starts, per)
    res = run_bass_kernel_spmd(nc, in_maps, list(range(N_CORES)))
    out = np.empty((T, D_MODEL), dtype=np.float32)
    for e in range(N_EXPERTS):
        out[e * per:(e + 1) * per] = np.asarray(res.results[e]["out"],
                                                dtype=np.float32)
    return out


# revision 17
# speedup vs baseline: 1.0051x; 1.0051x over previous
"""Grouped-expert SwiGLU FFN (MoE) kernel for 8 Trainium2 NeuronCores.

Problem: 8 experts, tokens pre-sorted into contiguous equal segments.
  sorted_x: (8192, 512) f32, w12: (8, 2048, 512) f32, w3: (8, 512, 1024) f32
  out[t] = SwiGLU(x[t] @ w12[e].T) @ w3[e].T   for t in expert e's segment

Sharding: expert parallelism. Core e gets its 1024-token segment plus
w12[e]/w3[e]; no collectives. Host pre-transposes operands so the
contraction dim lands on SBUF partitions (no on-chip transposes):

  GEMM1 (contract d_model=512, 4 chunks of 128):
    lhsT = w12[e].T chunk (i=128, j=128)  [stationary]
    rhs  = x_seg.T  chunk (i=128, t=512)  [moving]
    psum (j=128, t=512) accumulated over 4 i-chunks  -> h12^T orientation
  SwiGLU: silu(h1^T) * h2^T elementwise in (j, t) layout (ACT + DVE)
  GEMM2 (contract hidden=1024, 8 chunks of 128):
    lhsT = h^T chunk   (j=128, t=128)  [stationary]
    rhs  = w3[e].T chunk (j=128, o=512) [moving]
    psum (t=128, o=512) accumulated over 8 j-chunks -> natural (t, o) output

All operands are cast to fp16 on the host: halves HBM traffic vs f32 and
enables Fast Weight Load (disabled for fp32 dtypes), so LDWEIGHTS hides
under the 512-col matmul streams. PSUM accumulation stays fp32. Inputs
arrive in 6 host-packed partition-major DMAs ordered by consumption so
the first matmul is gated by only ~768 KB (w12 pair 0 + x token-block 0)
instead of the whole 4 MB. GEMM2 token-chunks are interleaved between
the two GEMM1 token-block passes so the SwiGLU ACT/DVE latency of one
block hides under the other block's matmuls. The output is stored fp16
(host upcasts) to shorten the store tail.
"""

import numpy as np

N_EXPERTS = 8
D_MODEL = 512
HIDDEN = 1024
TOKENS_PER_EXPERT = 1024
N_CORES = 8

_CACHE = {}


def _build_program(warm_mms=17):
    import concourse.mybir as mybir
    import concourse.tile as tile
    from concourse import bacc

    f32 = mybir.dt.float32
    f16 = mybir.dt.float16
    P = 128
    IC = D_MODEL // P            # 4 contraction chunks of d_model
    JC = HIDDEN // P             # 8 pair-chunks of 2*hidden
    TB = 512                     # token block (moving free dim / psum bank)
    NT = TOKENS_PER_EXPERT // TB  # 2 token blocks
    TM = TOKENS_PER_EXPERT // P  # 8 output token chunks

    nc = bacc.Bacc(None, target_bir_lowering=False)

    # host-packed partition-major layouts (see _pack_inputs)
    xt_d = nc.dram_tensor("xt", [NT, P, IC, TB], f16, kind="ExternalInput")
    w12a_d = nc.dram_tensor("w12a", [P, 2, IC * P], f16, kind="ExternalInput")
    w12b_d = nc.dram_tensor("w12b", [P, JC - 1, 2, IC * P], f16, kind="ExternalInput")
    w3_d = nc.dram_tensor("w3g", [P, JC, D_MODEL], f16, kind="ExternalInput")
    out_d = nc.dram_tensor("out", [TOKENS_PER_EXPERT, D_MODEL], f16,
                           kind="ExternalOutput")

    with tile.TileContext(nc) as tc:
        with (
            tc.tile_pool(name="persist", bufs=1) as persist,
            tc.tile_pool(name="work", bufs=3) as work,
            tc.tile_pool(name="ps1", bufs=2, space="PSUM") as ps1,
            tc.tile_pool(name="ps2", bufs=2, space="PSUM") as ps2,
        ):
            # ---- PE warm-up: zero matmuls during the initial DMA window
            # release the HAM clock gate before real work arrives.
            # memset on GpSimd (free right after the preamble) so the
            # dummy matmuls start ~6.3us, not after DVE's late start. ----
            if warm_mms:
                wz = persist.tile([P, 256], f16, tag="warmz", name="warmz")
                nc.gpsimd.memset(wz[:], 0.0)
                wps = ps2.tile([P, 256], f32, tag="warmps", bufs=1, name="warmps")
                for i in range(warm_mms):
                    nc.tensor.matmul(wps[:], wz[:, 0:P], wz[:], start=True,
                                     stop=True)

            # ---- loads, in consumption order. The SP HWDGE ring's
            # per-queue FIFO keeps later loads from stealing bandwidth
            # ahead of their turn; the small gating w12 pair-0 load rides
            # the ACT ring in parallel. ----
            w12a = persist.tile([P, 2, IC * P], f16, tag="w12a", name="w12a")
            nc.scalar.dma_start(w12a[:], w12a_d[:])
            xt = [persist.tile([P, IC, TB], f16, tag=f"xt{tb}", name=f"xt{tb}") for tb in range(NT)]
            nc.sync.dma_start(xt[0][:, 0:2, :], xt_d[0, :, 0:2])
            nc.sync.dma_start(xt[0][:, 2:4, :], xt_d[0, :, 2:4])
            w12b = persist.tile([P, JC - 1, 2, IC * P], f16, tag="w12b", name="w12b")
            nc.sync.dma_start(w12b[:, 0], w12b_d[:, 0])
            nc.sync.dma_start(w12b[:, 1], w12b_d[:, 1])
            nc.sync.dma_start(w12b[:, 2:4], w12b_d[:, 2:4])
            nc.sync.dma_start(w12b[:, 4:6], w12b_d[:, 4:6])
            nc.sync.dma_start(w12b[:, 6], w12b_d[:, 6])
            nc.sync.dma_start(xt[1][:], xt_d[1])
            w3 = persist.tile([P, JC, D_MODEL], f16, tag="w3", name="w3")
            nc.sync.dma_start(w3[:], w3_d[:])

            def w12sel(p, h, ic):
                sl = slice(ic * P, (ic + 1) * P)
                if p == 0:
                    return w12a[:, h, sl]
                return w12b[:, p - 1, h, sl]

            h = [persist.tile([P, TOKENS_PER_EXPERT], f16, tag=f"h{p}", name=f"h{p}")
                 for p in range(JC)]

            def gemm1(tb):
                for p in range(JC):
                    ps_a = ps1.tile([P, TB], f32, tag="ps_a", name=f"a{p}_{tb}")
                    ps_b = ps1.tile([P, TB], f32, tag="ps_b", name=f"b{p}_{tb}")
                    for ic in range(IC):
                        nc.tensor.matmul(ps_a[:], w12sel(p, 0, ic),
                                         xt[tb][:, ic, :],
                                         start=(ic == 0), stop=(ic == IC - 1))
                    for ic in range(IC):
                        nc.tensor.matmul(ps_b[:], w12sel(p, 1, ic),
                                         xt[tb][:, ic, :],
                                         start=(ic == 0), stop=(ic == IC - 1))
                    s = work.tile([P, TB], f32, tag="silu", name=f"s{p}_{tb}")
                    nc.scalar.activation(s[:], ps_a[:],
                                         mybir.ActivationFunctionType.Silu)
                    nc.vector.tensor_tensor(h[p][:, tb * TB:(tb + 1) * TB],
                                            s[:], ps_b[:], mybir.AluOpType.mult)

            def gemm2(tm, split=1):
                ps_o = ps2.tile([P, D_MODEL], f32, tag="ps_o", name=f"o{tm}")
                for jc in range(JC):
                    nc.tensor.matmul(ps_o[:], h[jc][:, tm * P:(tm + 1) * P],
                                     w3[:, jc, :],
                                     start=(jc == 0), stop=(jc == JC - 1))
                o = work.tile([P, D_MODEL], f16, tag="o", name=f"oo{tm}")
                # split the final chunk's cast+store across both HWDGE
                # rings so the exposed receipt chain after the last matmul
                # is as short as possible
                w = D_MODEL // split
                for sp in range(split):
                    sl = slice(sp * w, (sp + 1) * w)
                    nc.vector.tensor_copy(o[:, sl], ps_o[:, sl])
                    eng = nc.scalar if sp % 2 else nc.sync
                    eng.dma_start(out_d[tm * P:(tm + 1) * P, sl], o[:, sl])

            gemm1(0)
            gemm1(1)
            for tm in range(TM):
                gemm2(tm, split=2 if tm == TM - 1 else 1)

    nc.compile()
    return nc


def _pack_inputs(sorted_x, w12, w3, starts, per):
    """Host-side shard + transpose + fp16 packing for each core."""
    in_maps = []
    for e in range(N_EXPERTS):
        xs = sorted_x[starts[e]:starts[e] + per]                 # (1024, 512)
        # xt[tb, p, ic, t] = xs[tb*512 + t, ic*128 + p]
        xt = np.ascontiguousarray(
            xs.reshape(2, 512, 4, 128).transpose(0, 3, 2, 1)
        ).astype(np.float16)
        # w12c[q][k][ic*128+j] = w12[e][q*128+j, ic*128+k]
        w12c = w12[e].reshape(16, 128, 4, 128).transpose(0, 3, 2, 1).reshape(
            16, 128, 512)
        # pair-major: pair p = (chunk p [h1], chunk p+8 [h2])
        w12p = np.stack([w12c[:8], w12c[8:]], axis=1)            # (8, 2, 128, 512)
        w12a = np.ascontiguousarray(
            w12p[0].transpose(1, 0, 2)).astype(np.float16)       # (128, 2, 512)
        w12b = np.ascontiguousarray(
            w12p[1:].transpose(2, 0, 1, 3)).astype(np.float16)   # (128, 7, 2, 512)
        # w3g[p][jc][o] = w3[e][o, jc*128+p]
        w3g = np.ascontiguousarray(
            w3[e].reshape(512, 8, 128).transpose(2, 1, 0)
        ).astype(np.float16)                                     # (128, 8, 512)
        in_maps.append({"xt": xt, "w12a": w12a, "w12b": w12b, "w3g": w3g})
    return in_maps


def _reference_numpy(sorted_x, w12, w3, expert_starts, expert_ends):
    """Exact fallback for non-canonical segment layouts."""
    x = sorted_x.astype(np.float32)
    T = x.shape[0]
    out = np.zeros((T, w3.shape[1]), dtype=np.float32)
    tok = np.arange(T)
    for e in range(w12.shape[0]):
        m = (tok >= expert_starts[e]) & (tok < expert_ends[e])
        if not m.any():
            continue
        h12 = x[m] @ w12[e].T
        h1, h2 = h12[:, :HIDDEN], h12[:, HIDDEN:]
        hact = (h1 / (1.0 + np.exp(-h1))) * h2
        out[m] += hact @ w3[e].T
    return out


def kernel(sorted_x, w12, w3, expert_starts, expert_ends):
    sorted_x = np.asarray(sorted_x)
    w12 = np.asarray(w12)
    w3 = np.asarray(w3)
    starts = np.asarray(expert_starts).astype(np.int64)
    ends = np.asarray(expert_ends).astype(np.int64)

    T = sorted_x.shape[0]
    E = w12.shape[0]
    per = T // E
    canonical = (
        E == N_EXPERTS
        and T == N_EXPERTS * TOKENS_PER_EXPERT
        and sorted_x.shape[1] == D_MODEL
        and w12.shape[1:] == (2 * HIDDEN, D_MODEL)
        and w3.shape[1:] == (D_MODEL, HIDDEN)
        and bool(np.all(starts == np.arange(E, dtype=np.int64) * per))
        and bool(np.all(ends == starts + per))
    )
    if not canonical:
        return _reference_numpy(sorted_x, w12, w3, starts, ends)

    from concourse.bass_utils import run_bass_kernel_spmd

    if "nc" not in _CACHE:
        _CACHE["nc"] = _build_program()
    nc = _CACHE["nc"]

    in_maps = _pack_inputs(sorted_x, w12, w3, starts, per)
    res = run_bass_kernel_spmd(nc, in_maps, list(range(N_CORES)))
    out = np.empty((T, D_MODEL), dtype=np.float32)
    for e in range(N_EXPERTS):
        out[e * per:(e + 1) * per] = np.asarray(res.results[e]["out"],
                                                dtype=np.float32)
    return out


# revision 18
# speedup vs baseline: 1.0198x; 1.0146x over previous
"""Grouped-expert SwiGLU FFN (MoE) kernel for 8 Trainium2 NeuronCores.

Problem: 8 experts, tokens pre-sorted into contiguous equal segments.
  sorted_x: (8192, 512) f32, w12: (8, 2048, 512) f32, w3: (8, 512, 1024) f32
  out[t] = SwiGLU(x[t] @ w12[e].T) @ w3[e].T   for t in expert e's segment

Sharding: expert parallelism. Core e gets its 1024-token segment plus
w12[e]/w3[e]; no collectives. Host pre-transposes operands so the
contraction dim lands on SBUF partitions (no on-chip transposes):

  GEMM1 (contract d_model=512, 4 chunks of 128):
    lhsT = w12[e].T chunk (i=128, j=128)  [stationary]
    rhs  = x_seg.T  chunk (i=128, t=512)  [moving]
    psum (j=128, t=512) accumulated over 4 i-chunks  -> h12^T orientation
  SwiGLU: silu(h1^T) * h2^T elementwise in (j, t) layout (ACT + DVE)
  GEMM2 (contract hidden=1024, 8 chunks of 128):
    lhsT = h^T chunk   (j=128, t=128)  [stationary]
    rhs  = w3[e].T chunk (j=128, o=512) [moving]
    psum (t=128, o=512) accumulated over 8 j-chunks -> natural (t, o) output

All operands are cast to fp16 on the host: halves HBM traffic vs f32 and
enables Fast Weight Load (disabled for fp32 dtypes), so LDWEIGHTS hides
under the 512-col matmul streams. PSUM accumulation stays fp32. Inputs
arrive in 6 host-packed partition-major DMAs ordered by consumption so
the first matmul is gated by only ~768 KB (w12 pair 0 + x token-block 0)
instead of the whole 4 MB. GEMM2 token-chunks are interleaved between
the two GEMM1 token-block passes so the SwiGLU ACT/DVE latency of one
block hides under the other block's matmuls. The output is stored fp16
(host upcasts) to shorten the store tail.
"""

import numpy as np

N_EXPERTS = 8
D_MODEL = 512
HIDDEN = 1024
TOKENS_PER_EXPERT = 1024
N_CORES = 8

_CACHE = {}


def _build_program(warm_mms=17):
    import concourse.mybir as mybir
    import concourse.tile as tile
    from concourse import bacc

    f32 = mybir.dt.float32
    f16 = mybir.dt.float16
    P = 128
    IC = D_MODEL // P            # 4 contraction chunks of d_model
    JC = HIDDEN // P             # 8 pair-chunks of 2*hidden
    TB = 512                     # token block (moving free dim / psum bank)
    NT = TOKENS_PER_EXPERT // TB  # 2 token blocks
    TM = TOKENS_PER_EXPERT // P  # 8 output token chunks

    nc = bacc.Bacc(None, target_bir_lowering=False)

    # host-packed partition-major layouts (see _pack_inputs)
    xt_d = nc.dram_tensor("xt", [NT, P, IC, TB], f16, kind="ExternalInput")
    w12a_d = nc.dram_tensor("w12a", [P, 2, IC * P], f16, kind="ExternalInput")
    w12b_d = nc.dram_tensor("w12b", [P, JC - 1, 2, IC * P], f16, kind="ExternalInput")
    w3_d = nc.dram_tensor("w3g", [P, JC, D_MODEL], f16, kind="ExternalInput")
    out_d = nc.dram_tensor("out", [TOKENS_PER_EXPERT, D_MODEL], f16,
                           kind="ExternalOutput")

    with tile.TileContext(nc) as tc:
        with (
            tc.tile_pool(name="persist", bufs=1) as persist,
            tc.tile_pool(name="work", bufs=3) as work,
            tc.tile_pool(name="ps1", bufs=2, space="PSUM") as ps1,
            tc.tile_pool(name="ps2", bufs=2, space="PSUM") as ps2,
        ):
            # ---- PE warm-up: zero matmuls during the initial DMA window
            # release the HAM clock gate before real work arrives.
            # memset on GpSimd (free right after the preamble) so the
            # dummy matmuls start ~6.3us, not after DVE's late start. ----
            if warm_mms:
                wz = persist.tile([P, 256], f16, tag="warmz", name="warmz")
                nc.gpsimd.memset(wz[:], 0.0)
                wps = ps2.tile([P, 256], f32, tag="warmps", bufs=1, name="warmps")
                for i in range(warm_mms):
                    nc.tensor.matmul(wps[:], wz[:, 0:P], wz[:], start=True,
                                     stop=True)

            # ---- loads, in consumption order. The SP HWDGE ring's
            # per-queue FIFO keeps later loads from stealing bandwidth
            # ahead of their turn; the small gating w12 pair-0 load rides
            # the ACT ring in parallel. ----
            w12a = persist.tile([P, 2, IC * P], f16, tag="w12a", name="w12a")
            nc.scalar.dma_start(w12a[:], w12a_d[:])
            xt = [persist.tile([P, IC, TB], f16, tag=f"xt{tb}", name=f"xt{tb}") for tb in range(NT)]
            nc.sync.dma_start(xt[0][:, 0:2, :], xt_d[0, :, 0:2])
            nc.sync.dma_start(xt[0][:, 2:4, :], xt_d[0, :, 2:4])
            w12b = persist.tile([P, JC - 1, 2, IC * P], f16, tag="w12b", name="w12b")
            nc.sync.dma_start(w12b[:, 0], w12b_d[:, 0])
            nc.sync.dma_start(w12b[:, 1], w12b_d[:, 1])
            nc.sync.dma_start(w12b[:, 2:4], w12b_d[:, 2:4])
            nc.sync.dma_start(w12b[:, 4:6], w12b_d[:, 4:6])
            nc.sync.dma_start(w12b[:, 6], w12b_d[:, 6])
            nc.sync.dma_start(xt[1][:], xt_d[1])
            w3 = persist.tile([P, JC, D_MODEL], f16, tag="w3", name="w3")
            nc.sync.dma_start(w3[:], w3_d[:])

            def w12sel(p, h, ic):
                sl = slice(ic * P, (ic + 1) * P)
                if p == 0:
                    return w12a[:, h, sl]
                return w12b[:, p - 1, h, sl]

            h = [persist.tile([P, TOKENS_PER_EXPERT], f16, tag=f"h{p}", name=f"h{p}")
                 for p in range(JC)]

            def gemm1(tb):
                for p in range(JC):
                    ps_a = ps1.tile([P, TB], f32, tag="ps_a", name=f"a{p}_{tb}")
                    ps_b = ps1.tile([P, TB], f32, tag="ps_b", name=f"b{p}_{tb}")
                    for ic in range(IC):
                        nc.tensor.matmul(ps_a[:], w12sel(p, 0, ic),
                                         xt[tb][:, ic, :],
                                         start=(ic == 0), stop=(ic == IC - 1))
                    for ic in range(IC):
                        nc.tensor.matmul(ps_b[:], w12sel(p, 1, ic),
                                         xt[tb][:, ic, :],
                                         start=(ic == 0), stop=(ic == IC - 1))
                    s = work.tile([P, TB], f32, tag="silu", name=f"s{p}_{tb}")
                    nc.scalar.activation(s[:], ps_a[:],
                                         mybir.ActivationFunctionType.Silu)
                    nc.vector.tensor_tensor(h[p][:, tb * TB:(tb + 1) * TB],
                                            s[:], ps_b[:], mybir.AluOpType.mult)

            def gemm2(tm, split=1):
                # split > 1 divides the psum group into column halves so
                # the final chunk's cast+store chain starts as soon as its
                # half of the matmuls lands, and the stores fan out over
                # both HWDGE rings — shortest exposed tail after the last
                # matmul.
                w = D_MODEL // split
                o = work.tile([P, D_MODEL], f16, tag="o", name=f"oo{tm}")
                for sp in range(split):
                    sl = slice(sp * w, (sp + 1) * w)
                    ps_o = ps2.tile([P, w], f32, tag="ps_o", name=f"o{tm}_{sp}")
                    for jc in range(JC):
                        nc.tensor.matmul(ps_o[:], h[jc][:, tm * P:(tm + 1) * P],
                                         w3[:, jc, sl],
                                         start=(jc == 0), stop=(jc == JC - 1))
                    nc.vector.tensor_copy(o[:, sl], ps_o[:])
                    eng = nc.scalar if sp % 2 else nc.sync
                    eng.dma_start(out_d[tm * P:(tm + 1) * P, sl], o[:, sl])

            gemm1(0)
            gemm1(1)
            for tm in range(TM):
                gemm2(tm, split=2 if tm == TM - 1 else 1)

    nc.compile()
    return nc


def _pack_inputs(sorted_x, w12, w3, starts, per):
    """Host-side shard + transpose + fp16 packing for each core."""
    in_maps = []
    for e in range(N_EXPERTS):
        xs = sorted_x[starts[e]:starts[e] + per]                 # (1024, 512)
        # xt[tb, p, ic, t] = xs[tb*512 + t, ic*128 + p]
        xt = np.ascontiguousarray(
            xs.reshape(2, 512, 4, 128).transpose(0, 3, 2, 1)
        ).astype(np.float16)
        # w12c[q][k][ic*128+j] = w12[e][q*128+j, ic*128+k]
        w12c = w12[e].reshape(16, 128, 4, 128).transpose(0, 3, 2, 1).reshape(
            16, 128, 512)
        # pair-major: pair p = (chunk p [h1], chunk p+8 [h2])
        w12p = np.stack([w12c[:8], w12c[8:]], axis=1)            # (8, 2, 128, 512)
        w12a = np.ascontiguousarray(
            w12p[0].transpose(1, 0, 2)).astype(np.float16)       # (128, 2, 512)
        w12b = np.ascontiguousarray(
            w12p[1:].transpose(2, 0, 1, 3)).astype(np.float16)   # (128, 7, 2, 512)
        # w3g[p][jc][o] = w3[e][o, jc*128+p]
        w3g = np.ascontiguousarray(
            w3[e].reshape(512, 8, 128).transpose(2, 1, 0)
        ).astype(np.float16)                                     # (128, 8, 512)
        in_maps.append({"xt": xt, "w12a": w12a, "w12b": w12b, "w3g": w3g})
    return in_maps


def _reference_numpy(sorted_x, w12, w3, expert_starts, expert_ends):
    """Exact fallback for non-canonical segment layouts."""
    x = sorted_x.astype(np.float32)
    T = x.shape[0]
    out = np.zeros((T, w3.shape[1]), dtype=np.float32)
    tok = np.arange(T)
    for e in range(w12.shape[0]):
        m = (tok >= expert_starts[e]) & (tok < expert_ends[e])
        if not m.any():
            continue
        h12 = x[m] @ w12[e].T
        h1, h2 = h12[:, :HIDDEN], h12[:, HIDDEN:]
        hact = (h1 / (1.0 + np.exp(-h1))) * h2
        out[m] += hact @ w3[e].T
    return out


def kernel(sorted_x, w12, w3, expert_starts, expert_ends):
    sorted_x = np.asarray(sorted_x)
    w12 = np.asarray(w12)
    w3 = np.asarray(w3)
    starts = np.asarray(expert_starts).astype(np.int64)
    ends = np.asarray(expert_ends).astype(np.int64)

    T = sorted_x.shape[0]
    E = w12.shape[0]
    per = T // E
    canonical = (
        E == N_EXPERTS
        and T == N_EXPERTS * TOKENS_PER_EXPERT
        and sorted_x.shape[1] == D_MODEL
        and w12.shape[1:] == (2 * HIDDEN, D_MODEL)
        and w3.shape[1:] == (D_MODEL, HIDDEN)
        and bool(np.all(starts == np.arange(E, dtype=np.int64) * per))
        and bool(np.all(ends == starts + per))
    )
    if not canonical:
        return _reference_numpy(sorted_x, w12, w3, starts, ends)

    from concourse.bass_utils import run_bass_kernel_spmd

    if "nc" not in _CACHE:
        _CACHE["nc"] = _build_program()
    nc = _CACHE["nc"]

    in_maps = _pack_inputs(sorted_x, w12, w3, starts, per)
    res = run_bass_kernel_spmd(nc, in_maps, list(range(N_CORES)))
    out = np.empty((T, D_MODEL), dtype=np.float32)
    for e in range(N_EXPERTS):
        out[e * per:(e + 1) * per] = np.asarray(res.results[e]["out"],
                                                dtype=np.float32)
    return out
